# revision 12
# baseline (speedup 1.0000x reference)
"""Trainium2 Bass kernel for nn_DotProductAttention_76338748719461.

Attention with a multiplicative mask and softmax over the QUERY axis
(axis=1 of [B, Lq, Lk] scores):

    S[b,q,k]  = (Q[b,q,:] . K[b,k,:]) / 8 + max(log(mask[0,q,k]), F32_MIN)
    A         = softmax(S, axis=q)
    out[b,q,v]= sum_k A[b,q,k] * V[b,k,v]

Per-core layout (batch data-parallel over 8 cores, 2 batches/core):

  * Host pre-transposes Q/8 and K into [128, 2048] tiles (batch 0 on
    partitions 0-63, batch 1 on 64-127) and ships ln(mask).T as f16 --
    all transposes and the log leave the device entirely.
  * Scores are built TRANSPOSED, S_T[k, q], so the softmax reduction
    (over q) is the ACT accumulator's free-axis sum.
  * ln(mask) bias is added into PSUM by two concurrent 64x64
    identity matmuls (tile_position'd to disjoint quadrants), then the
    QK matmuls accumulate on top: S' = lnm + K.T Q / 8.
  * ACT does exp(S') PSUM->SBUF f16 with accum_out giving the softmax
    denominator D per k-row for free.
  * AV: O_T[v, q] += (V/D).T @ PM, col-packed so batch 0 lands in PSUM
    partitions 0-63 and batch 1 in 64-127 (one 4-bank accumulator).
  * Output written as O_T [2, 64, 2048]; host transposes back.

PSUM budget: O accumulator 4 banks + two [128, 1024] score slots
(4 banks) = 8 banks exactly.

Optional: SCHRAUD_COLS>0 offloads that many columns of the last unit
of each k-tile to a DVE Schraudolph exp (bf16 bit-trick, ~3% max err
on those columns) to relieve ACT. Default 0 (exact).
"""

import os
import numpy as np

B, LQ, LK, D, DV = 16, 2048, 2048, 64, 64
NCORES = 8
BPC = B // NCORES  # batches per core
P = 128
NT_K = LK // P  # 16 k-tiles
HALF = 1024  # score-slot width (2 PSUM banks)
SCALE = 1.0 / 8.0

MAIN_REPS = int(os.environ.get("MAIN_REPS", "1"))  # repeat body (timing builds)
SCHRAUD_COLS = int(os.environ.get("SCHRAUD_COLS", "0"))  # DVE exp offload
LNM_CLAMP = -60000.0  # keeps f16 finite; exp()==0 exactly either way

# Schraudolph bf16 constants: i16 = rint(A*x + B); bitcast bf16 ~= exp(x).
SCH_A = 128.0 / np.log(2.0)
SCH_B = 16256.0 - 7.0  # zero weighted-mean error calibration

_CACHED = None


def _emit_body(nc, tc, ctx, aps, dts):
    import concourse.mybir as mybir
    from concourse.bass import ds
    from concourse.masks import make_identity

    qt_d, kt_d, v_d, lnm_d, o_d = aps
    f32, f16, bf16, i16, f32r, AF, ALU = dts

    consts = ctx.enter_context(tc.tile_pool(name="consts", bufs=1))
    ident16 = consts.tile([P, P], f16)
    make_identity(nc, ident16)

    big = ctx.enter_context(tc.tile_pool(name="big", bufs=1))
    QT32 = big.tile([P, LQ], f32)  # rows 0-63: b0 dims (already /8), 64-127: b1
    KT32 = big.tile([P, LK], f32)
    QT = big.tile([P, LQ], f32r)
    KT = big.tile([P, LK], f32r)
    v_nat = big.tile([P, BPC, NT_K, DV], f32)
    nc.sync.dma_start(QT32[:], qt_d[:])
    nc.sync.dma_start(KT32[:], kt_d[:])
    nc.sync.dma_start(v_nat[:], v_d.rearrange("b (t p) d -> p b t d", p=P))
    # PE requires f32r operands pre-rounded; fine-grained so QK(j=0) starts
    # as soon as its chunks land
    from concourse.bass import ds as _ds
    for c in range(4):
        nc.vector.tensor_copy(KT[:, _ds(512 * c, 512)], KT32[:, _ds(512 * c, 512)])
    for c in range(4):
        nc.vector.tensor_copy(QT[:, _ds(512 * c, 512)], QT32[:, _ds(512 * c, 512)])

    lnm_pool = ctx.enter_context(tc.tile_pool(name="lnm", bufs=4))
    pm_pool = ctx.enter_context(tc.tile_pool(name="pm", bufs=2))
    s_pool = ctx.enter_context(tc.tile_pool(name="s", bufs=2, space="PSUM"))
    o_pool = ctx.enter_context(tc.tile_pool(name="o", bufs=1, space="PSUM"))
    work = ctx.enter_context(tc.tile_pool(name="work", bufs=2))
    outp = ctx.enter_context(tc.tile_pool(name="outp", bufs=1))

    for _mr in range(MAIN_REPS):
        _emit_pass(nc, tc, dts, ident16, QT, KT, v_nat,
                   lnm_pool, pm_pool, s_pool, o_pool, work, outp,
                   lnm_d, o_d)


def _emit_pass(nc, tc, dts, ident16, QT, KT, v_nat,
               lnm_pool, pm_pool, s_pool, o_pool, work, outp, lnm_d, o_d):
    import concourse.mybir as mybir
    from concourse.bass import ds, ts

    f32, f16, bf16, i16, f32r, AF, ALU = dts

    O_ps = o_pool.tile([P, LQ], f32, tag="o", name="O_ps")

    # prefetch first ln-mask panels
    lnm_tiles = {}
    for j in range(min(3, NT_K)):
        t = lnm_pool.tile([P, LK], f16, tag="lnm", name=f"lnm{j}")
        nc.sync.dma_start(t[:], lnm_d[ds(P * j, P), :])
        lnm_tiles[j] = t

    pending_av = None
    for j in range(NT_K):
        if j + 3 < NT_K:
            jj = j + 3
            t = lnm_pool.tile([P, LK], f16, tag="lnm", name=f"lnm{jj}")
            nc.sync.dma_start(t[:], lnm_d[ds(P * jj, P), :])
            lnm_tiles[jj] = t
        lnm = lnm_tiles.pop(j)

        PM = [
            pm_pool.tile([P, LQ], f16, tag=f"pm{b}", name=f"PM{b}")
            for b in range(BPC)
        ]
        Dp = work.tile([P, BPC, 3], f32, tag="dp", name="Dp")

        # 4 units: (b, h) with batch-major h so slots ping-pong A/B
        for u, (b, h) in enumerate(((0, 0), (1, 0), (0, 1), (1, 1))):
            S = s_pool.tile([P, HALF], f32, tag="s", name=f"s{u}")
            # per 512-chunk: QK opens the accumulation group, then two
            # concurrent 64x64 identity matmuls add the lnm bias on top
            bs = ds(64 * b, 64)
            for c in range(HALF // 512):
                qs = ds(HALF * h + 512 * c, 512)
                nc.tensor.matmul(
                    S[:, ts(c, 512)],
                    KT[bs, ds(P * j, P)],
                    QT[bs, qs],
                    start=True, stop=True,
                )
                # group bookkeeping skipped: sim's zero-region tracker
                # drops base_partition on quadrant tiles; stop is sim-only
                nc.tensor.matmul(
                    S[0:64, ts(c, 512)], ident16[0:64, 0:64], lnm[0:64, qs],
                    start=False, stop=False, skip_group_check=True,
                )
                nc.tensor.matmul(
                    S[64:128, ts(c, 512)], ident16[64:128, 64:128],
                    lnm[64:128, qs],
                    start=False, stop=False, skip_group_check=True,
                )
            # exp: ACT with fused row-sum; optional DVE offload of the
            # tail columns of the last unit
            sch = SCHRAUD_COLS if u == 3 else 0
            act_w = HALF - sch
            if act_w > 0:
                nc.scalar.activation(
                    PM[b][:, ds(HALF * h, act_w)], S[:, 0:act_w], AF.Exp,
                    accum_out=Dp[:, b, ds(h, 1)],
                )
            if sch > 0:
                pm_bits = PM[b][:, ds(HALF * h + act_w, sch)].bitcast(i16)
                nc.vector.tensor_scalar(
                    pm_bits, S[:, ds(act_w, sch)], SCH_A, SCH_B,
                    ALU.mult, ALU.add,
                )
                pm_b16 = PM[b][:, ds(HALF * h + act_w, sch)].bitcast(bf16)
                nc.vector.tensor_scalar(
                    pm_b16, pm_b16, 1.0, 0.0, ALU.mult, ALU.add,
                    accum_out=Dp[:, b, ds(2, 1)],
                )

        if pending_av is not None:
            _emit_av(nc, O_ps, pending_av)

        Vp = work.tile([P, BPC, DV], f16, tag="vp", name="Vp")
        for b in range(BPC):
            Db = work.tile([P, 1], f32, tag=f"d{b}", name=f"D{b}")
            nparts = 3 if (SCHRAUD_COLS and b == 1) else 2
            nc.vector.reduce_sum(
                Db[:], Dp[:, b, 0:nparts], axis=mybir.AxisListType.X
            )
            Rb = work.tile([P, 1], f32, tag=f"r{b}", name=f"R{b}")
            nc.vector.reciprocal(Rb[:], Db[:])
            nc.gpsimd.tensor_scalar_mul(Vp[:, b, :], v_nat[:, b, j, :], Rb[:])
        pending_av = (Vp, PM, j)

    _emit_av(nc, O_ps, pending_av)

    # epilogue: copy PSUM -> SBUF, DMA out transposed layout [b, v, q]
    O_sb = outp.tile([P, LQ], f32, tag="osb", name="O_sb")
    for c in range(2):
        nc.vector.tensor_copy(O_sb[:, ts(c, HALF)], O_ps[:, ts(c, HALF)])
        for b in range(BPC):
            nc.sync.dma_start(
                o_d[b, :, ts(c, HALF)], O_sb[ds(64 * b, 64), ts(c, HALF)]
            )


def _emit_av(nc, O_ps, pending):
    from concourse.bass import ts

    Vp, PM, j = pending
    for c in range(LQ // 512):
        for b in range(len(PM)):
            nc.tensor.matmul(
                O_ps[ts(b, 64), ts(c, 512)],
                Vp[:, b, :],
                PM[b][:, ts(c, 512)],
                start=(j == 0), stop=(j == NT_K - 1),
                skip_group_check=(b > 0),
            )


def _build_module():
    import concourse.mybir as mybir
    import concourse.tile as tile
    from concourse import bacc
    from contextlib import ExitStack

    f32 = mybir.dt.float32
    f16 = mybir.dt.float16
    bf16 = mybir.dt.bfloat16
    i16 = mybir.dt.int16
    f32r = mybir.dt.float32r
    dts = (f32, f16, bf16, i16, f32r,
           mybir.ActivationFunctionType, mybir.AluOpType)

    nc = bacc.Bacc("TRN2", target_bir_lowering=False, debug=False)
    qt_d = nc.dram_tensor("qt", [P, LQ], f32, kind="ExternalInput").ap()
    kt_d = nc.dram_tensor("kt", [P, LK], f32, kind="ExternalInput").ap()
    v_d = nc.dram_tensor("v", [BPC, LK, DV], f32, kind="ExternalInput").ap()
    lnm_d = nc.dram_tensor("lnm", [LK, LQ], f16, kind="ExternalInput").ap()
    o_d = nc.dram_tensor("o", [BPC, DV, LQ], f32, kind="ExternalOutput").ap()
    aps = (qt_d, kt_d, v_d, lnm_d, o_d)

    with tile.TileContext(nc) as tc:
        with ExitStack() as rctx:
            _emit_body(nc, tc, rctx, aps, dts)

    nc.compile()
    return nc


def _get_module():
    global _CACHED
    if _CACHED is None:
        _CACHED = _build_module()
    return _CACHED


def _prep_host(query, key, value, mask):
    """Host-side reformatting: transposes, scaling, log(mask)."""
    query = np.asarray(query, dtype=np.float32)
    key = np.asarray(key, dtype=np.float32)
    value = np.asarray(value, dtype=np.float32)
    mask = np.asarray(mask, dtype=np.float32)

    # [NCORES, 128, LQ]: per core, rows 0-63 = batch0 dims, 64-127 = batch1
    qs = (query * np.float32(SCALE)).reshape(NCORES, BPC, LQ, D)
    qt = np.ascontiguousarray(qs.transpose(0, 1, 3, 2)).reshape(NCORES, P, LQ)
    ks = key.reshape(NCORES, BPC, LK, D)
    kt = np.ascontiguousarray(ks.transpose(0, 1, 3, 2)).reshape(NCORES, P, LK)
    vs = value.reshape(NCORES, BPC, LK, DV)

    with np.errstate(divide="ignore"):
        lnm = np.log(mask[0])
    lnm = np.maximum(lnm, LNM_CLAMP)
    lnmT = np.ascontiguousarray(lnm.T).astype(np.float16)  # [LK, LQ]
    return qt, kt, vs, lnmT


def kernel(query, key, value, mask, _trace=False):
    from concourse.bass_utils import run_bass_kernel_spmd

    qt, kt, vs, lnmT = _prep_host(query, key, value, mask)

    nc = _get_module()
    in_maps = [
        {"qt": qt[c], "kt": kt[c], "v": vs[c], "lnm": lnmT}
        for c in range(NCORES)
    ]
    res = run_bass_kernel_spmd(
        nc, in_maps, core_ids=list(range(NCORES)), trace=_trace
    )
    # o: [BPC, DV, LQ] per core -> [B, LQ, DV]
    o_t = np.stack([res.results[c]["o"] for c in range(NCORES)])  # [NC,BPC,DV,LQ]
    out = np.ascontiguousarray(o_t.transpose(0, 1, 3, 2)).reshape(B, LQ, DV)
    if _trace:
        return out, res
    return out
